# revision 6
# baseline (speedup 1.0000x reference)
"""Multi-head attention (B=4, S=2048, D=1024, H=16) on 8 TRN2 NeuronCores.

Sharding: core cid handles batch b = cid//2 and head-group hg = cid%2
(8 heads = 512 channels).  Each core computes, for its (b, hg):
  QT = (Wq_hg/8) @ q[b].T + bq/8      [512, 2048]  (channels on partitions)
  KT = Wk_hg @ k[b].T + bk            [512, 2048]
  V  = v[b] @ Wv_hg.T + bv            [2048, 512]  (seq on partitions)
  per head: scoresT = KT_h^T-blocks @ QT_h (contraction d_k=64, two heads
  packed in the 128-partition dim via PE row groups), softmax over the
  partition (S_k) axis computed WITHOUT max-subtraction (scores are O(10);
  exp gets a constant -12 bias that cancels in the normalization) with the
  row-sum obtained for free from a ones-column appended to V,
  attnT accumulated over S_k chunks in PSUM, normalized by the
  PE-broadcast reciprocal of the sums row, written into concatT.
  yT_partial = Wo_hg-rows.T-contraction @ concatT   [1024, 2048]
Host sums the two head-group partials per batch, transposes, adds bo.

All PE operands are fp16 (tf32-class mantissa for these magnitudes);
accumulation is fp32 in PSUM.
"""
import numpy as np
from contextlib import ExitStack

import concourse.bass as bass
import concourse.tile as tile
import concourse.mybir as mybir
import concourse.bass_utils as bass_utils

D_MODEL = 1024
NHEAD = 16
D_K = 64
B = 4
S = 2048
N_CORES = 8
HG = 8            # heads per core
C = HG * D_K      # 512 channels per core
P = 128
EXP_BIAS = -12.0

F16 = mybir.dt.float16
F32 = mybir.dt.float32


def _split_waits(nc, max_waits=1):
    """Cayman CTRL/LW instruction structs carry a single sync-wait slot and
    this walrus rejects instructions with more; move excess SyncWaits onto
    injected same-engine NOPs placed immediately before the instruction."""
    n = 0
    for fn in nc.m.functions:
        for bb in fn.blocks:
            insts = list(bb.instructions)
            out = []
            changed = False
            for inst in insts:
                si = inst.sync_info
                waits = list(si.on_wait) if si is not None and si.on_wait else []
                if len(waits) > max_waits:
                    changed = True
                    extra, keep = waits[:-max_waits], waits[-max_waits:]
                    for w in extra:
                        n += 1
                        nop = mybir.InstNoOp(name=f"wsplit_{n}", ins=[], outs=[])
                        nop.engine = inst.engine
                        nop.sync_info = mybir.SyncInfo(on_wait=[w], on_update=[])
                        out.append(nop)
                    inst.sync_info = mybir.SyncInfo(
                        on_wait=keep,
                        on_update=list(si.on_update) if si.on_update else [],
                    )
                out.append(inst)
            if changed:
                bb.instructions = out
    return n


def build_program(S_=S):
    NSB = S_ // 512    # 512-wide seq blocks
    NSC = S_ // P      # 128-wide seq chunks
    ND = D_MODEL // P  # model-dim chunks (contraction for projections)
    NCC = C // P       # channel chunks = head pairs

    nc = bass.Bass("TRN2", target_bir_lowering=False, debug=False,
                   num_devices=N_CORES)
    dt_in = F16
    xq = nc.dram_tensor("xqT", [D_MODEL, S_], dt_in, kind="ExternalInput").ap()
    xk = nc.dram_tensor("xkT", [D_MODEL, S_], dt_in, kind="ExternalInput").ap()
    xv = nc.dram_tensor("xvT", [D_MODEL, S_], dt_in, kind="ExternalInput").ap()
    wq = nc.dram_tensor("wqT", [D_MODEL, C], dt_in, kind="ExternalInput").ap()
    wk = nc.dram_tensor("wkT", [D_MODEL, C], dt_in, kind="ExternalInput").ap()
    wv = nc.dram_tensor("wvT", [D_MODEL, C], dt_in, kind="ExternalInput").ap()
    wo = nc.dram_tensor("woT", [C, D_MODEL], dt_in, kind="ExternalInput").ap()
    bq = nc.dram_tensor("bq", [1, C], dt_in, kind="ExternalInput").ap()
    bk = nc.dram_tensor("bk", [1, C], dt_in, kind="ExternalInput").ap()
    bv = nc.dram_tensor("bv", [1, C], dt_in, kind="ExternalInput").ap()
    yTa = nc.dram_tensor("yTa", [D_MODEL, S_], F32, kind="ExternalOutput").ap()
    yTb = nc.dram_tensor("yTb", [D_MODEL, S_], F32, kind="ExternalOutput").ap()

    with tile.TileContext(nc) as tc, ExitStack() as ctx:
        const = ctx.enter_context(tc.tile_pool(name="const", bufs=1))
        big = ctx.enter_context(tc.tile_pool(name="big", bufs=1))
        wpool = ctx.enter_context(tc.tile_pool(name="wp", bufs=1))
        xpool = ctx.enter_context(tc.tile_pool(name="xp", bufs=10))
        epool = ctx.enter_context(tc.tile_pool(name="ep", bufs=6))
        spool = ctx.enter_context(tc.tile_pool(name="sp", bufs=3))
        psum = ctx.enter_context(tc.tile_pool(name="ps", bufs=1, space="PSUM"))

        ones = const.tile([1, 512], F16, tag="ones")
        nc.vector.memset(ones[:], 1.0)
        ebias = const.tile([P, 1], F32, tag="ebias")
        nc.vector.memset(ebias[:], EXP_BIAS)
        bq_sb = const.tile([1, C], F16, tag="bq")
        nc.sync.dma_start(bq_sb[:], bq)
        bk_sb = const.tile([1, C], F16, tag="bk")
        nc.sync.dma_start(bk_sb[:], bk)
        bv_sb = const.tile([1, C], F16, tag="bv")
        nc.sync.dma_start(bv_sb[:], bv)

        QT = big.tile([P, NCC, S_], F16, tag="QT")
        KT = big.tile([P, NCC, S_], F16, tag="KT")
        V = big.tile([P, NSC, HG, 66], F16, tag="V")
        CT = big.tile([P, NCC, S_], F16, tag="CT")
        nc.vector.memset(V[:, :, :, 64:65], 1.0)

        Exp = mybir.ActivationFunctionType.Exp
        mult = mybir.AluOpType.mult

        # ---- V projection inputs: wv + xvT stay resident ----
        wv_sb = wpool.tile([P, ND, C], F16, tag="wv", name="w_v")
        nc.sync.dma_start(wv_sb[:], wv.rearrange("(c p) m -> p c m", p=P))
        xv_res = big.tile([P, ND, S_], F16, tag="xv")
        for dc in range(ND):
            nc.sync.dma_start(xv_res[:, dc, :], xv[dc * P:(dc + 1) * P, :])

        def proj_v(p):
            """V columns for head pair p only (N=128), so attention(p) can
            start as early as possible."""
            cs = slice(p * P, (p + 1) * P)
            for sc in range(NSC):
                pt = psum.tile([P, 512], F32, tag="proj", bufs=2,
                               name=f"pv_{p}_{sc}")
                nc.tensor.matmul(pt[:, 0:P], ones[0:1, 0:P], bv_sb[0:1, cs],
                                 start=True, stop=False)
                for dc in range(ND):
                    nc.tensor.matmul(pt[:, 0:P],
                                     xv_res[:, dc, sc * P:(sc + 1) * P],
                                     wv_sb[:, dc, cs], start=False,
                                     stop=(dc == ND - 1))
                nc.vector.tensor_copy(
                    V[:, sc, 2 * p:2 * p + 2, 0:64],
                    pt[:, 0:P].rearrange("p (h d) -> p h d", h=2))

        # ---- Q/K projections per head-pair chunk, then attention ----
        def proj_qk(p):
            for name, wd, b_sb, out_t in (("q", wq, bq_sb, QT),
                                          ("k", wk, bk_sb, KT)):
                for sb_ in range(NSB):
                    xts = []
                    xd = xq if name == "q" else xk
                    for dc in range(ND):
                        xt = xpool.tile([P, 512], F16, tag="xt",
                                        name=f"x{name}_{p}_{sb_}_{dc}")
                        nc.sync.dma_start(
                            xt[:],
                            xd[dc * P:(dc + 1) * P, sb_ * 512:(sb_ + 1) * 512])
                        xts.append(xt)
                    pt = psum.tile([P, 512], F32, tag="proj", bufs=2,
                                   name=f"p{name}_{p}_{sb_}")
                    nc.tensor.matmul(pt[:], b_sb[0:1, p * P:(p + 1) * P],
                                     ones[0:1, :], start=True, stop=False)
                    for dc in range(ND):
                        nc.tensor.matmul(pt[:],
                                         wd_sb[name][:, dc, p * P:(p + 1) * P],
                                         xts[dc][:], start=False,
                                         stop=(dc == ND - 1))
                    nc.vector.tensor_copy(
                        out_t[:, p, sb_ * 512:(sb_ + 1) * 512], pt[:])

        # weights for q/k stay resident (they are reused across all pairs)
        wd_sb = {}
        for name, wd in (("q", wq), ("k", wk)):
            t = wpool.tile([P, ND, C], F16, tag=f"w{name}", name=f"w_{name}")
            nc.sync.dma_start(t[:], wd.rearrange("(c p) m -> p c m", p=P))
            wd_sb[name] = t

        def attention(p):
            for sq in range(NSB):
                qs = slice(sq * 512, (sq + 1) * 512)
                atA = psum.tile([P, 512], F32, tag="attn", bufs=2, name=f"atA_{p}_{sq}")
                atB = psum.tile([P, 512], F32, tag="attn", bufs=2, name=f"atB_{p}_{sq}")
                for skp in range(NSC // 2):
                    k0 = slice(2 * skp * P, (2 * skp + 1) * P)
                    k1 = slice((2 * skp + 1) * P, (2 * skp + 2) * P)
                    sA = psum.tile([P, 1024], F32, tag="sc", bufs=2, name=f"sA_{p}_{sq}_{skp}")
                    sB = psum.tile([P, 1024], F32, tag="sc", bufs=2, name=f"sB_{p}_{sq}_{skp}")
                    nc.tensor.matmul(sA[:, 0:512], KT[0:64, p, k0],
                                     QT[0:64, p, qs], start=True, stop=True)
                    nc.tensor.matmul(sA[:, 512:1024], KT[0:64, p, k1],
                                     QT[0:64, p, qs], start=True, stop=True)
                    nc.tensor.matmul(sB[:, 0:512], KT[64:128, p, k0],
                                     QT[64:128, p, qs], start=True, stop=True)
                    nc.tensor.matmul(sB[:, 512:1024], KT[64:128, p, k1],
                                     QT[64:128, p, qs], start=True, stop=True)
                    eA = epool.tile([P, 1024], F16, tag="exp", name=f"eA_{p}_{sq}_{skp}")
                    nc.scalar.activation(eA[:], sA[:], Exp, bias=ebias[:])
                    eB = epool.tile([P, 1024], F16, tag="exp", name=f"eB_{p}_{sq}_{skp}")
                    nc.scalar.activation(eB[:], sB[:], Exp, bias=ebias[:])
                    nc.tensor.matmul(atA[0:65], V[:, 2 * skp, 2 * p, 0:65],
                                     eA[:, 0:512], start=(skp == 0), stop=False)
                    nc.tensor.matmul(atA[0:65], V[:, 2 * skp + 1, 2 * p, 0:65],
                                     eA[:, 512:1024], start=False,
                                     stop=(skp == NSC // 2 - 1))
                    nc.tensor.matmul(atB[0:65], V[:, 2 * skp, 2 * p + 1, 0:65],
                                     eB[:, 0:512], start=(skp == 0), stop=False)
                    nc.tensor.matmul(atB[0:65], V[:, 2 * skp + 1, 2 * p + 1, 0:65],
                                     eB[:, 512:1024], start=False,
                                     stop=(skp == NSC // 2 - 1))
                for hh, at in ((0, atA), (1, atB)):
                    inv = spool.tile([1, 512], F16, tag="inv",
                                     name=f"inv_{p}_{sq}_{hh}")
                    with nc.allow_low_precision(
                            reason="softmax 1/sum in fp16: uniform per-column "
                                   "scale, ~3e-4 rel err is within budget"):
                        nc.vector.reciprocal(inv[:], at[64:65, :])
                    bi = psum.tile([P, 1024], F32, tag="sc", bufs=2,
                                   name=f"bi_{p}_{sq}_{hh}")
                    nc.tensor.matmul(bi[0:64, 0:512], ones[0:1, 0:64],
                                     inv[0:1, :], start=True, stop=True)
                    bis = spool.tile([64, 512], F16, tag="bis",
                                     name=f"bis_{p}_{sq}_{hh}")
                    nc.vector.tensor_copy(bis[:], bi[0:64, 0:512])
                    if hh == 0:
                        nc.vector.tensor_tensor(CT[0:64, p, qs], at[0:64, :],
                                                bis[:], mult)
                    else:
                        tmp = spool.tile([64, 512], F16, tag="tmpB",
                                         name=f"tmpB_{p}_{sq}")
                        nc.vector.tensor_tensor(tmp[:], at[0:64, :], bis[:], mult)
                        nc.sync.dma_start(CT[64:128, p, qs], tmp[:])

        for p in range(NCC):
            proj_v(p)
            proj_qk(p)
            attention(p)

        # ---- output projection: yT = woT-chunks^T-contract @ CT ----
        wo_sb = wpool.tile([P, NCC, D_MODEL], F16, tag="wo", name="w_o")
        nc.sync.dma_start(wo_sb[:], wo.rearrange("(c p) m -> p c m", p=P))
        for half, yT_d in ((0, yTa), (1, yTb)):
            for mc in range(ND):
                ms = slice(mc * P, (mc + 1) * P)
                for sb_ in range(NSB):
                    ss = slice(sb_ * 512, (sb_ + 1) * 512)
                    pt = psum.tile([P, 512], F32, tag="proj", bufs=2,
                                   name=f"py_{half}_{mc}_{sb_}")
                    for i, pcc in enumerate((2 * half, 2 * half + 1)):
                        nc.tensor.matmul(pt[:], wo_sb[:, pcc, ms],
                                         CT[:, pcc, ss], start=(i == 0),
                                         stop=(i == 1))
                    st = spool.tile([P, 512], F32, tag="stage",
                                    name=f"st_{half}_{mc}_{sb_}")
                    nc.vector.tensor_copy(st[:], pt[:])
                    nc.sync.dma_start(yT_d[ms, ss], st[:])

    _split_waits(nc, max_waits=1)
    return nc


_PROGRAM = None


def _get_program():
    global _PROGRAM
    if _PROGRAM is None:
        _PROGRAM = build_program()
    return _PROGRAM


def _make_in_maps(q, k, v, Wq, bq, Wk, bk, Wv, bv, Wo, bo):
    f16 = np.float16
    xqT = [np.ascontiguousarray(q[b].T, dtype=f16) for b in range(B)]
    xkT = [np.ascontiguousarray(k[b].T, dtype=f16) for b in range(B)]
    xvT = [np.ascontiguousarray(v[b].T, dtype=f16) for b in range(B)]
    WqT = np.ascontiguousarray(Wq.T * 0.125, dtype=f16)
    WkT = np.ascontiguousarray(Wk.T, dtype=f16)
    WvT = np.ascontiguousarray(Wv.T, dtype=f16)
    WoT = np.ascontiguousarray(Wo.T, dtype=f16)
    in_maps = []
    for cid in range(N_CORES):
        b, hg = divmod(cid, 2)
        sl = slice(hg * C, (hg + 1) * C)
        in_maps.append({
            "xqT": xqT[b], "xkT": xkT[b], "xvT": xvT[b],
            "wqT": np.ascontiguousarray(WqT[:, sl]),
            "wkT": np.ascontiguousarray(WkT[:, sl]),
            "wvT": np.ascontiguousarray(WvT[:, sl]),
            "woT": np.ascontiguousarray(WoT[sl, :]),
            "bq": (bq[sl] * 0.125).astype(f16).reshape(1, C),
            "bk": bk[sl].astype(f16).reshape(1, C),
            "bv": bv[sl].astype(f16).reshape(1, C),
        })
    return in_maps


def run(inputs, trace=False, trace_cores=None):
    nc = _get_program()
    in_maps = _make_in_maps(**{k: np.asarray(v) for k, v in inputs.items()})
    res = bass_utils.run_bass_kernel_spmd(
        nc, in_maps, core_ids=list(range(N_CORES)), trace=trace,
        trace_cores=trace_cores)
    bo = np.asarray(inputs["bo"], dtype=np.float64)
    out = np.empty((B, S, D_MODEL), np.float32)
    for b in range(B):
        acc = (res.results[2 * b]["yTa"].astype(np.float64)
               + res.results[2 * b]["yTb"].astype(np.float64)
               + res.results[2 * b + 1]["yTa"].astype(np.float64)
               + res.results[2 * b + 1]["yTb"].astype(np.float64)).T + bo
        out[b] = acc.astype(np.float32)
    return out, res


def kernel(**inputs):
    out, _ = run(inputs, trace=False)
    return out


# revision 8
# speedup vs baseline: 162.3752x; 162.3752x over previous
"""Multi-head attention (B=4, S=2048, D=1024, H=16) on 8 TRN2 NeuronCores.

Sharding: core cid handles batch b = cid//2 and head-group hg = cid%2
(8 heads = 512 channels).  Each core computes, for its (b, hg):
  QT = (Wq_hg/8) @ q[b].T + bq/8      [512, 2048]  (channels on partitions)
  KT = Wk_hg @ k[b].T + bk            [512, 2048]
  V  = v[b] @ Wv_hg.T + bv            [2048, 512]  (seq on partitions)
  per head: scoresT = KT_h^T-blocks @ QT_h (contraction d_k=64, two heads
  packed in the 128-partition dim via PE row groups), softmax over the
  partition (S_k) axis computed WITHOUT max-subtraction (scores are O(10);
  exp gets a constant -12 bias that cancels in the normalization) with the
  row-sum obtained for free from a ones-column appended to V,
  attnT accumulated over S_k chunks in PSUM, normalized by the
  PE-broadcast reciprocal of the sums row, written into concatT.
  yT_partial = Wo_hg-rows.T-contraction @ concatT   [1024, 2048]
Host sums the two head-group partials per batch, transposes, adds bo.

All PE operands are fp16 (tf32-class mantissa for these magnitudes);
accumulation is fp32 in PSUM.
"""
import numpy as np
from contextlib import ExitStack, nullcontext

import concourse.bass as bass
import concourse.tile as tile
import concourse.mybir as mybir
import concourse.bass_utils as bass_utils

D_MODEL = 1024
NHEAD = 16
D_K = 64
B = 4
S = 2048
N_CORES = 8
HG = 8            # heads per core
C = HG * D_K      # 512 channels per core
P = 128
EXP_BIAS = -12.0

F16 = mybir.dt.float16
F32 = mybir.dt.float32


def _split_waits(nc, max_waits=1):
    """Cayman CTRL/LW instruction structs carry a single sync-wait slot and
    this walrus rejects instructions with more; move excess SyncWaits onto
    injected same-engine NOPs placed immediately before the instruction."""
    n = 0
    for fn in nc.m.functions:
        for bb in fn.blocks:
            insts = list(bb.instructions)
            out = []
            changed = False
            for inst in insts:
                si = inst.sync_info
                waits = list(si.on_wait) if si is not None and si.on_wait else []
                if len(waits) > max_waits:
                    changed = True
                    extra, keep = waits[:-max_waits], waits[-max_waits:]
                    for w in extra:
                        n += 1
                        nop = mybir.InstNoOp(name=f"wsplit_{n}", ins=[], outs=[])
                        nop.engine = inst.engine
                        nop.sync_info = mybir.SyncInfo(on_wait=[w], on_update=[])
                        out.append(nop)
                    inst.sync_info = mybir.SyncInfo(
                        on_wait=keep,
                        on_update=list(si.on_update) if si.on_update else [],
                    )
                out.append(inst)
            if changed:
                bb.instructions = out
    return n


def build_program(S_=S, reps=1):
    NSB = S_ // 512    # 512-wide seq blocks
    NSC = S_ // P      # 128-wide seq chunks
    ND = D_MODEL // P  # model-dim chunks (contraction for projections)
    NCC = C // P       # channel chunks = head pairs

    nc = bass.Bass("TRN2", target_bir_lowering=False, debug=False,
                   num_devices=N_CORES)
    dt_in = F16
    xq = nc.dram_tensor("xqT", [D_MODEL, S_], dt_in, kind="ExternalInput").ap()
    xk = nc.dram_tensor("xkT", [D_MODEL, S_], dt_in, kind="ExternalInput").ap()
    xv = nc.dram_tensor("xvT", [D_MODEL, S_], dt_in, kind="ExternalInput").ap()
    wq = nc.dram_tensor("wqT", [D_MODEL, C], dt_in, kind="ExternalInput").ap()
    wk = nc.dram_tensor("wkT", [D_MODEL, C], dt_in, kind="ExternalInput").ap()
    wv = nc.dram_tensor("wvT", [D_MODEL, C], dt_in, kind="ExternalInput").ap()
    wo = nc.dram_tensor("woT", [C, D_MODEL], dt_in, kind="ExternalInput").ap()
    bq = nc.dram_tensor("bq", [1, C], dt_in, kind="ExternalInput").ap()
    bk = nc.dram_tensor("bk", [1, C], dt_in, kind="ExternalInput").ap()
    bv = nc.dram_tensor("bv", [1, C], dt_in, kind="ExternalInput").ap()
    yTa = nc.dram_tensor("yTa", [D_MODEL, S_], F32, kind="ExternalOutput").ap()
    yTb = nc.dram_tensor("yTb", [D_MODEL, S_], F32, kind="ExternalOutput").ap()

    with tile.TileContext(nc) as tc, ExitStack() as ctx:
        const = ctx.enter_context(tc.tile_pool(name="const", bufs=1))
        big = ctx.enter_context(tc.tile_pool(name="big", bufs=1))
        wpool = ctx.enter_context(tc.tile_pool(name="wp", bufs=1))
        xpool = ctx.enter_context(tc.tile_pool(name="xp", bufs=10))
        epool = ctx.enter_context(tc.tile_pool(name="ep", bufs=6))
        spool = ctx.enter_context(tc.tile_pool(name="sp", bufs=3))
        psum = ctx.enter_context(tc.tile_pool(name="ps", bufs=1, space="PSUM"))

        ones = const.tile([1, 512], F16, tag="ones")
        nc.vector.memset(ones[:], 1.0)
        ebias = const.tile([P, 1], F32, tag="ebias")
        nc.vector.memset(ebias[:], EXP_BIAS)
        bq_sb = const.tile([1, C], F16, tag="bq")
        nc.sync.dma_start(bq_sb[:], bq)
        bk_sb = const.tile([1, C], F16, tag="bk")
        nc.sync.dma_start(bk_sb[:], bk)
        bv_sb = const.tile([1, C], F16, tag="bv")
        nc.sync.dma_start(bv_sb[:], bv)

        QT = big.tile([P, NCC, S_], F16, tag="QT")
        KT = big.tile([P, NCC, S_], F16, tag="KT")
        V = big.tile([P, NSC, HG, 66], F16, tag="V")
        CT = big.tile([P, NCC, S_], F16, tag="CT")
        nc.vector.memset(V[:, :, :, 64:65], 1.0)

        Exp = mybir.ActivationFunctionType.Exp
        mult = mybir.AluOpType.mult

        # ---- V projection inputs: wv + xvT stay resident ----
        wv_sb = wpool.tile([P, ND, C], F16, tag="wv", name="w_v")
        nc.sync.dma_start(wv_sb[:], wv.rearrange("(c p) m -> p c m", p=P))
        xv_res = big.tile([P, ND, S_], F16, tag="xv")
        for dc in range(ND):
            nc.sync.dma_start(xv_res[:, dc, :], xv[dc * P:(dc + 1) * P, :])

        def proj_v(p):
            """V columns for head pair p only (N=128), so attention(p) can
            start as early as possible."""
            cs = slice(p * P, (p + 1) * P)
            for sc in range(NSC):
                pt = psum.tile([P, 512], F32, tag="proj", bufs=2,
                               name=f"pv_{p}_{sc}")
                nc.tensor.matmul(pt[:, 0:P], ones[0:1, 0:P], bv_sb[0:1, cs],
                                 start=True, stop=False)
                for dc in range(ND):
                    nc.tensor.matmul(pt[:, 0:P],
                                     xv_res[:, dc, sc * P:(sc + 1) * P],
                                     wv_sb[:, dc, cs], start=False,
                                     stop=(dc == ND - 1))
                nc.vector.tensor_copy(
                    V[:, sc, 2 * p:2 * p + 2, 0:64],
                    pt[:, 0:P].rearrange("p (h d) -> p h d", h=2))

        # ---- Q/K projections per head-pair chunk, then attention ----
        def proj_qk(p):
            for name, wd, b_sb, out_t in (("q", wq, bq_sb, QT),
                                          ("k", wk, bk_sb, KT)):
                for sb_ in range(NSB):
                    xts = []
                    xd = xq if name == "q" else xk
                    for dc in range(ND):
                        xt = xpool.tile([P, 512], F16, tag="xt",
                                        name=f"x{name}_{p}_{sb_}_{dc}")
                        nc.sync.dma_start(
                            xt[:],
                            xd[dc * P:(dc + 1) * P, sb_ * 512:(sb_ + 1) * 512])
                        xts.append(xt)
                    pt = psum.tile([P, 512], F32, tag="proj", bufs=2,
                                   name=f"p{name}_{p}_{sb_}")
                    nc.tensor.matmul(pt[:], b_sb[0:1, p * P:(p + 1) * P],
                                     ones[0:1, :], start=True, stop=False)
                    for dc in range(ND):
                        nc.tensor.matmul(pt[:],
                                         wd_sb[name][:, dc, p * P:(p + 1) * P],
                                         xts[dc][:], start=False,
                                         stop=(dc == ND - 1))
                    nc.vector.tensor_copy(
                        out_t[:, p, sb_ * 512:(sb_ + 1) * 512], pt[:])

        # weights for q/k stay resident (they are reused across all pairs)
        wd_sb = {}
        for name, wd in (("q", wq), ("k", wk)):
            t = wpool.tile([P, ND, C], F16, tag=f"w{name}", name=f"w_{name}")
            nc.sync.dma_start(t[:], wd.rearrange("(c p) m -> p c m", p=P))
            wd_sb[name] = t
        wo_sb = wpool.tile([P, NCC, D_MODEL], F16, tag="wo", name="w_o")
        nc.sync.dma_start(wo_sb[:], wo.rearrange("(c p) m -> p c m", p=P))

        def attention(p):
            for sq in range(NSB):
                qs = slice(sq * 512, (sq + 1) * 512)
                atA = psum.tile([P, 512], F32, tag="attn", bufs=2, name=f"atA_{p}_{sq}")
                atB = psum.tile([P, 512], F32, tag="attn", bufs=2, name=f"atB_{p}_{sq}")
                for skp in range(NSC // 2):
                    k0 = slice(2 * skp * P, (2 * skp + 1) * P)
                    k1 = slice((2 * skp + 1) * P, (2 * skp + 2) * P)
                    sA = psum.tile([P, 1024], F32, tag="sc", bufs=2, name=f"sA_{p}_{sq}_{skp}")
                    sB = psum.tile([P, 1024], F32, tag="sc", bufs=2, name=f"sB_{p}_{sq}_{skp}")
                    nc.tensor.matmul(sA[:, 0:512], KT[0:64, p, k0],
                                     QT[0:64, p, qs], start=True, stop=True)
                    nc.tensor.matmul(sA[:, 512:1024], KT[0:64, p, k1],
                                     QT[0:64, p, qs], start=True, stop=True)
                    nc.tensor.matmul(sB[:, 0:512], KT[64:128, p, k0],
                                     QT[64:128, p, qs], start=True, stop=True)
                    nc.tensor.matmul(sB[:, 512:1024], KT[64:128, p, k1],
                                     QT[64:128, p, qs], start=True, stop=True)
                    eA = epool.tile([P, 1024], F16, tag="exp", name=f"eA_{p}_{sq}_{skp}")
                    nc.scalar.activation(eA[:], sA[:], Exp, bias=ebias[:])
                    eB = epool.tile([P, 1024], F16, tag="exp", name=f"eB_{p}_{sq}_{skp}")
                    nc.scalar.activation(eB[:], sB[:], Exp, bias=ebias[:])
                    nc.tensor.matmul(atA[0:65], V[:, 2 * skp, 2 * p, 0:65],
                                     eA[:, 0:512], start=(skp == 0), stop=False)
                    nc.tensor.matmul(atA[0:65], V[:, 2 * skp + 1, 2 * p, 0:65],
                                     eA[:, 512:1024], start=False,
                                     stop=(skp == NSC // 2 - 1))
                    nc.tensor.matmul(atB[0:65], V[:, 2 * skp, 2 * p + 1, 0:65],
                                     eB[:, 0:512], start=(skp == 0), stop=False)
                    nc.tensor.matmul(atB[0:65], V[:, 2 * skp + 1, 2 * p + 1, 0:65],
                                     eB[:, 512:1024], start=False,
                                     stop=(skp == NSC // 2 - 1))
                for hh, at in ((0, atA), (1, atB)):
                    inv = spool.tile([1, 512], F16, tag="inv",
                                     name=f"inv_{p}_{sq}_{hh}")
                    with nc.allow_low_precision(
                            reason="softmax 1/sum in fp16: uniform per-column "
                                   "scale, ~3e-4 rel err is within budget"):
                        nc.vector.reciprocal(inv[:], at[64:65, :])
                    bi = psum.tile([P, 1024], F32, tag="sc", bufs=2,
                                   name=f"bi_{p}_{sq}_{hh}")
                    nc.tensor.matmul(bi[0:64, 0:512], ones[0:1, 0:64],
                                     inv[0:1, :], start=True, stop=True)
                    bis = spool.tile([64, 512], F16, tag="bis",
                                     name=f"bis_{p}_{sq}_{hh}")
                    nc.vector.tensor_copy(bis[:], bi[0:64, 0:512])
                    if hh == 0:
                        nc.vector.tensor_tensor(CT[0:64, p, qs], at[0:64, :],
                                                bis[:], mult)
                    else:
                        tmp = spool.tile([64, 512], F16, tag="tmpB",
                                         name=f"tmpB_{p}_{sq}")
                        nc.vector.tensor_tensor(tmp[:], at[0:64, :], bis[:], mult)
                        nc.sync.dma_start(CT[64:128, p, qs], tmp[:])

        loop_cm = tc.For_i(0, reps, 1) if reps > 1 else nullcontext()
        with loop_cm:
            for p in range(NCC):
                proj_v(p)
                proj_qk(p)
                attention(p)

            # ---- output projection: yT = woT-chunks^T-contract @ CT ----
            for half, yT_d in ((0, yTa), (1, yTb)):
                for mc in range(ND):
                    ms = slice(mc * P, (mc + 1) * P)
                    for sb_ in range(NSB):
                        ss = slice(sb_ * 512, (sb_ + 1) * 512)
                        pt = psum.tile([P, 512], F32, tag="proj", bufs=2,
                                       name=f"py_{half}_{mc}_{sb_}")
                        for i, pcc in enumerate((2 * half, 2 * half + 1)):
                            nc.tensor.matmul(pt[:], wo_sb[:, pcc, ms],
                                             CT[:, pcc, ss], start=(i == 0),
                                             stop=(i == 1))
                        st = spool.tile([P, 512], F32, tag="stage",
                                        name=f"st_{half}_{mc}_{sb_}")
                        nc.vector.tensor_copy(st[:], pt[:])
                        nc.sync.dma_start(yT_d[ms, ss], st[:])

    _split_waits(nc, max_waits=1)
    return nc


_PROGRAM = None


def _get_program():
    global _PROGRAM
    if _PROGRAM is None:
        _PROGRAM = build_program()
    return _PROGRAM


def _make_in_maps(q, k, v, Wq, bq, Wk, bk, Wv, bv, Wo, bo):
    f16 = np.float16
    xqT = [np.ascontiguousarray(q[b].T, dtype=f16) for b in range(B)]
    xkT = [np.ascontiguousarray(k[b].T, dtype=f16) for b in range(B)]
    xvT = [np.ascontiguousarray(v[b].T, dtype=f16) for b in range(B)]
    WqT = np.ascontiguousarray(Wq.T * 0.125, dtype=f16)
    WkT = np.ascontiguousarray(Wk.T, dtype=f16)
    WvT = np.ascontiguousarray(Wv.T, dtype=f16)
    WoT = np.ascontiguousarray(Wo.T, dtype=f16)
    in_maps = []
    for cid in range(N_CORES):
        b, hg = divmod(cid, 2)
        sl = slice(hg * C, (hg + 1) * C)
        in_maps.append({
            "xqT": xqT[b], "xkT": xkT[b], "xvT": xvT[b],
            "wqT": np.ascontiguousarray(WqT[:, sl]),
            "wkT": np.ascontiguousarray(WkT[:, sl]),
            "wvT": np.ascontiguousarray(WvT[:, sl]),
            "woT": np.ascontiguousarray(WoT[sl, :]),
            "bq": (bq[sl] * 0.125).astype(f16).reshape(1, C),
            "bk": bk[sl].astype(f16).reshape(1, C),
            "bv": bv[sl].astype(f16).reshape(1, C),
        })
    return in_maps


def run(inputs, trace=False, trace_cores=None):
    nc = _get_program()
    in_maps = _make_in_maps(**{k: np.asarray(v) for k, v in inputs.items()})
    res = bass_utils.run_bass_kernel_spmd(
        nc, in_maps, core_ids=list(range(N_CORES)), trace=trace,
        trace_cores=trace_cores)
    bo = np.asarray(inputs["bo"], dtype=np.float64)
    out = np.empty((B, S, D_MODEL), np.float32)
    for b in range(B):
        acc = (res.results[2 * b]["yTa"].astype(np.float64)
               + res.results[2 * b]["yTb"].astype(np.float64)
               + res.results[2 * b + 1]["yTa"].astype(np.float64)
               + res.results[2 * b + 1]["yTb"].astype(np.float64)).T + bo
        out[b] = acc.astype(np.float32)
    return out, res


def kernel(**inputs):
    out, _ = run(inputs, trace=False)
    return out
